# revision 24
# baseline (speedup 1.0000x reference)
# CantorAttention Trainium2 kernel.
#
# Math (see the original nn.Module): qkv projection of x [1, 4096, 1024],
# per-head sparse attention where each query attends to its 64 nearest
# neighbors in 1-D cantor-coordinate space, then an output projection.
#
# Key structural trick: the 64 nearest neighbors of a point on a line form a
# contiguous window in coordinate-sorted order.  So after sorting tokens by
# cantor coordinate (host side), the sparse attention becomes *banded*
# attention: query at sorted position r attends to a 64-wide window inside
# [r-63, r+63].  We block queries into 128-row tiles; each tile's windows all
# fit inside a 256-wide band of keys ([B-64, B+192) for query block base B).
#
# Sharding: the 4096 sorted tokens are split across 8 cores (512 each), with a
# 64-row halo on both sides (zero-padded at the global edges).  Each core
# computes q/k/v projections for its slice locally (weights replicated), does
# banded attention for all 16 heads, and the output projection for its rows.
# No collectives needed; host concatenates + un-sorts the result.
#
# Attention is computed with the *scores transposed*: s_T[tk, tq] comes from
# swapping the matmul operands (lhsT = k^T chunk, rhs = q^T block), so the
# probabilities are already in the [tk, tq] layout the attn@v matmul needs as
# its moving operand -- no PE transposes, no prob copies.  The mask is a
# binary multiply AFTER exp (exp of unmasked finite scores times 0), the
# softmax denominator z comes for free as a 65th ones-column appended to each
# head's v block (AV matmul M=65: rows 0:64 = unnormalized out, row 64 = z),
# and normalization multiplies by 1/z broadcast over each head's 64 feature
# rows via a tiny selector matmul (bsel [16,128] @ zinv [16,512]).  The v-bias
# is folded into b_out on the host (attn rows sum to 1).

import numpy as np
import ml_dtypes

N = 4096
DIM = 1024
HEADS = 16
HD = 64
KN = 64
SCALE = 1.0 / 8.0  # 1/sqrt(64)
NCORES = 8
SPC = N // NCORES          # 512 tokens owned per core
PAD = 64                   # halo rows on each side
HALO = SPC + 2 * PAD       # 640 rows of x each core sees
QB = 128                   # query block rows
NQB = SPC // QB            # 4 query blocks per core
KW = 2 * QB                # 256-wide key band per query block
NCH = HALO // 128          # 5 key chunks of 128 (halo space)
VW = HD + 1                # 65 columns per head in the augmented v tile

BF16 = ml_dtypes.bfloat16

LAST_RESULTS = None  # BassKernelResults of the most recent run (for test.py)


def _host_prep(x, cantor_coords, W_qkv, b_qkv, W_out, b_out):
    """Sort tokens by coordinate, build transposed banded masks, shard."""
    c = np.asarray(cantor_coords, dtype=np.float32)
    assert c.shape == (N,)

    # Exact replication of jax.lax.top_k(-d, 64): k smallest distances,
    # ties broken toward the lower index -> stable argsort on d.
    d = np.abs(c[:, None] - c[None, :])  # float32 [N, N]
    order = np.argsort(d, axis=1, kind="stable")[:, :KN]  # [N, KN]

    p = np.argsort(c, kind="stable")  # sorted order: sorted pos r -> orig idx
    pos = np.empty(N, dtype=np.int64)
    pos[p] = np.arange(N)

    # neighbors of the query at sorted position r, in sorted positions
    rs = pos[order[p]]  # [N, KN]
    B = (np.arange(N) // QB) * QB
    col = rs - B[:, None] + PAD  # column inside the 256-wide band
    assert col.min() >= 0 and col.max() < KW, "banded-window assumption broken"

    xs = np.asarray(x, dtype=np.float32)[0][p]  # [N, DIM] sorted
    xpad = np.zeros((N + 2 * PAD, DIM), dtype=np.float32)
    xpad[PAD : PAD + N] = xs
    xT = np.ascontiguousarray(xpad.T.astype(BF16))  # [DIM, N + 128]

    wqkv = np.ascontiguousarray(np.asarray(W_qkv, dtype=np.float32).astype(BF16))
    wout = np.ascontiguousarray(np.asarray(W_out, dtype=np.float32).astype(BF16))
    bq = np.asarray(b_qkv, np.float32)
    bqt = np.ascontiguousarray(bq[0:DIM].reshape(8, 128).T)        # [128, 8]
    bkt = np.ascontiguousarray(bq[DIM : 2 * DIM].reshape(8, 128).T)
    # v-bias folded into the output bias: rows of attn sum to 1, so
    # (ao/z + bv) @ Wout + bo == (ao/z) @ Wout + (bv @ Wout + bo).
    bo_folded = (
        np.asarray(b_out, np.float64)
        + np.asarray(bq[2 * DIM : 3 * DIM], np.float64)
        @ np.asarray(W_out, np.float64)
    ).astype(np.float32)
    bot = np.ascontiguousarray(bo_folded.reshape(8, 128).T)

    # Transposed binary masks, chunked: chunk c covers halo key positions
    # [128c, 128c+128) and query span tq = 128(c-1)+j for j in [0, 256).
    # maskT[c, pp, j] = 1 iff key 128c+pp is routed for query 128(c-1)+j.
    in_maps = []
    for t in range(NCORES):
        l = np.arange(SPC)                        # local sorted row
        i = SPC * t + l                           # global sorted row
        qb = l // QB
        hp = col[i] + 128 * qb[:, None]           # neighbor halo position
        ch = hp // 128                            # chunk index (qb or qb+1)
        pp = hp % 128
        j = l[:, None] - 128 * (ch - 1)
        maskT = np.zeros((NCH, 128, KW), dtype=np.float32)
        maskT[ch, pp, j] = 1.0
        in_maps.append(
            {
                "xT": np.ascontiguousarray(xT[:, SPC * t : SPC * t + HALO]),
                "wqkv": wqkv,
                "wout": wout,
                "bq": bqt,
                "bk": bkt,
                "bo": bot,
                "maskT": np.ascontiguousarray(
                    maskT.reshape(NCH * 128, KW).astype(BF16)
                ),
            }
        )
    return in_maps, p


def build_nc():
    """Build the per-core Bass program (identical on all 8 cores)."""
    import concourse.mybir as mybir
    import concourse.tile as tile
    from concourse import bacc
    from concourse.masks import make_identity

    FP32 = mybir.dt.float32
    BF = mybir.dt.bfloat16
    AF = mybir.ActivationFunctionType
    MUL = mybir.AluOpType.mult

    # Bacc (not raw Bass): its finalize runs the wait-splitting passes the
    # TRN2 codegen requires (<=1 sync wait per instruction).
    nc = bacc.Bacc("TRN2", target_bir_lowering=False, debug=False)

    xT_d = nc.declare_dram_parameter("xT", [DIM, HALO], BF, isOutput=False)
    wqkv_d = nc.declare_dram_parameter("wqkv", [DIM, 3 * DIM], BF, isOutput=False)
    wout_d = nc.declare_dram_parameter("wout", [DIM, DIM], BF, isOutput=False)
    bq_d = nc.declare_dram_parameter("bq", [128, 8], FP32, isOutput=False)
    bk_d = nc.declare_dram_parameter("bk", [128, 8], FP32, isOutput=False)
    bo_d = nc.declare_dram_parameter("bo", [128, 8], FP32, isOutput=False)
    maskT_d = nc.declare_dram_parameter("maskT", [NCH * 128, KW], BF, isOutput=False)
    out_d = nc.declare_dram_parameter("out", [DIM, SPC], BF, isOutput=True)

    with tile.TileContext(nc) as tc:
        with (
            tc.tile_pool(name="const", bufs=1) as const,
            tc.tile_pool(name="psum", bufs=2, space="PSUM") as psum,
        ):
            # ---- persistent SBUF tiles -------------------------------------
            xt_a = const.tile([128, 8 * HALO], BF, name="xt", tag="xt")
            wq_a = const.tile([128, 8 * DIM], BF, name="wq", tag="wq")
            wk_a = const.tile([128, 8 * DIM], BF, name="wk", tag="wk")
            wv_a = const.tile([128, 8 * DIM], BF, name="wv", tag="wv")
            wo_a = const.tile([128, 8 * DIM], BF, name="wo", tag="wo")
            xt = [xt_a[:, HALO * i : HALO * (i + 1)] for i in range(8)]
            wqs = [wq_a[:, DIM * i : DIM * (i + 1)] for i in range(8)]
            wks = [wk_a[:, DIM * i : DIM * (i + 1)] for i in range(8)]
            wvs = [wv_a[:, DIM * i : DIM * (i + 1)] for i in range(8)]
            wo = [wo_a[:, DIM * i : DIM * (i + 1)] for i in range(8)]
            bq = const.tile([128, 8], FP32, name="bq", tag="bq")
            bk = const.tile([128, 8], FP32, name="bk", tag="bk")
            bo = const.tile([128, 8], FP32, name="bo", tag="bo")
            mk_a = const.tile([128, NCH * KW], BF, name="mkT", tag="mkT")
            mkT = [mk_a[:, KW * i : KW * (i + 1)] for i in range(NCH)]
            ident = const.tile([128, 128], BF, name="ident", tag="ident")
            qT = [const.tile([128, SPC], BF, name=f"qT{i}", tag=f"qT{i}") for i in range(8)]
            kT = [const.tile([128, HALO], BF, name=f"kT{i}", tag=f"kT{i}") for i in range(8)]
            vtk = [const.tile([128, HEADS * VW], BF, name=f"vtk{i}", tag=f"vtk{i}") for i in range(NCH)]
            em = [
                [const.tile([128, 2 * KW], BF, name=f"em{c}_{pr}", tag=f"em{c}_{pr}") for pr in range(8)]
                for c in range(NCH)
            ]
            aoQ = [const.tile([128, DIM], BF, name=f"aoQ{i}", tag=f"aoQ{i}") for i in range(NQB)]
            aoT = [const.tile([128, SPC], BF, name=f"aoT{i}", tag=f"aoT{i}") for i in range(8)]
            zr = [const.tile([128, 1], FP32, name=f"zr{i}", tag=f"zr{i}") for i in range(4)]
            yT = [const.tile([128, SPC], BF, name=f"yT{i}", tag=f"yT{i}") for i in range(8)]

            # ---- loads -----------------------------------------------------
            # load order matters: q-phase inputs first so PE starts ASAP,
            # out-proj weights last (needed latest)
            make_identity(nc, ident[:])
            # batched multi-block loads: few big descriptors (sync-engine
            # issue time, ~0.5us each, otherwise starves the PE early on)
            xT3 = xT_d[:].rearrange("(b p) n -> p b n", p=128)
            wq3 = wqkv_d[:, 0:DIM].rearrange("(b p) n -> p b n", p=128)
            wk3 = wqkv_d[:, DIM : 2 * DIM].rearrange("(b p) n -> p b n", p=128)
            wv3 = wqkv_d[:, 2 * DIM : 3 * DIM].rearrange("(b p) n -> p b n", p=128)
            wo3 = wout_d[:].rearrange("(b p) n -> p b n", p=128)
            mk3 = maskT_d[:].rearrange("(b p) n -> p b n", p=128)
            xt3s = xt_a[:].rearrange("p (b n) -> p b n", n=HALO)
            wq3s = wq_a[:].rearrange("p (b n) -> p b n", n=DIM)
            wk3s = wk_a[:].rearrange("p (b n) -> p b n", n=DIM)
            wv3s = wv_a[:].rearrange("p (b n) -> p b n", n=DIM)
            wo3s = wo_a[:].rearrange("p (b n) -> p b n", n=DIM)
            mk3s = mk_a[:].rearrange("p (b n) -> p b n", n=KW)
            nc.sync.dma_start(bq[:], bq_d[:])
            nc.sync.dma_start(xt3s[:], xT3[:])
            for i in range(4):
                nc.sync.dma_start(
                    wq3s[:, 2 * i : 2 * (i + 1), :], wq3[:, 2 * i : 2 * (i + 1), :]
                )
            for i in range(2):
                nc.sync.dma_start(
                    wk3s[:, 4 * i : 4 * (i + 1), :], wk3[:, 4 * i : 4 * (i + 1), :]
                )
            nc.sync.dma_start(bk[:], bk_d[:])
            nc.sync.dma_start(bo[:], bo_d[:])
            for i in range(2):
                nc.sync.dma_start(
                    wv3s[:, 4 * i : 4 * (i + 1), :], wv3[:, 4 * i : 4 * (i + 1), :]
                )
            nc.sync.dma_start(mk3s[:], mk3[:])
            nc.sync.dma_start(wo3s[:], wo3[:])

            # ones columns of the augmented v tiles (65th column per head)
            for tb in range(NCH):
                nc.vector.memset(
                    vtk[tb][:].rearrange("p (h d) -> p h d", d=VW)[:, :, HD : HD + 1],
                    1.0,
                )

            # PE warm-up: ~4us of dummy matmuls while the input DMAs stream.
            # The PE clock-gates to 1.2 GHz until it has been busy ~3.4us;
            # this burst (which depends only on the identity tile) brings it
            # to 2.4 GHz before the first real projection matmul.
            warm = psum.tile([128, 128], FP32, name="warm", tag="ao", bufs=2)
            for _ in range(20):
                nc.tensor.matmul(warm[:], ident[:], ident[:], start=True, stop=True)

            # ---- q^T = Wq^T @ x^T (center 512 tokens), feature-major -------
            # kb-outer over 4 concurrent accumulators: the first matmul only
            # needs wqs[0], so PE starts as soon as that DMA lands.
            for g in range(2):
                pss = [
                    psum.tile([128, SPC], FP32, name=f"mmq{g}{i}", tag="mm", bufs=4)
                    for i in range(4)
                ]
                for kb in range(8):
                    for i in range(4):
                        cb = 4 * g + i
                        nc.tensor.matmul(
                            pss[i][:],
                            wqs[kb][:, 128 * cb : 128 * (cb + 1)],
                            xt[kb][:, PAD : PAD + SPC],
                            start=(kb == 0),
                            stop=(kb == 7),
                        )
                for i in range(4):
                    cb = 4 * g + i
                    if i % 2 == 1:
                        nc.vector.tensor_scalar_add(
                            qT[cb][:], pss[i][:], bq[:, cb : cb + 1]
                        )
                    else:
                        nc.scalar.activation(
                            qT[cb][:], pss[i][:], AF.Identity, bias=bq[:, cb : cb + 1]
                        )

            # ---- k^T over all 640 halo tokens, feature-major ---------------
            for cb in range(8):
                for tch in range(2):
                    ps = psum.tile([128, HALO // 2], FP32, name="mmk", tag="mm", bufs=4)
                    for kb in range(8):
                        nc.tensor.matmul(
                            ps[:],
                            wks[kb][:, 128 * cb : 128 * (cb + 1)],
                            xt[kb][:, 320 * tch : 320 * (tch + 1)],
                            start=(kb == 0),
                            stop=(kb == 7),
                        )
                    if tch == 1:
                        nc.vector.tensor_scalar_add(
                            kT[cb][:, 320 * tch : 320 * (tch + 1)],
                            ps[:],
                            bk[:, cb : cb + 1],
                        )
                    else:
                        nc.scalar.activation(
                            kT[cb][:, 320 * tch : 320 * (tch + 1)],
                            ps[:],
                            AF.Identity,
                            bias=bk[:, cb : cb + 1],
                        )

            # ---- v token-major over all 640 halo tokens (bias folded) ------
            # written strided into the augmented layout (65 cols per head)
            for tb in range(NCH):
                for cc in range(2):
                    ps = psum.tile([128, SPC], FP32, name="mmv", tag="mm", bufs=4)
                    for kb in range(8):
                        nc.tensor.matmul(
                            ps[:],
                            xt[kb][:, 128 * tb : 128 * (tb + 1)],
                            wvs[kb][:, 512 * cc : 512 * (cc + 1)],
                            start=(kb == 0),
                            stop=(kb == 7),
                        )
                    vdst = (
                        vtk[tb][:]
                        .rearrange("p (h d) -> p h d", d=VW)[
                            :, 8 * cc : 8 * (cc + 1), 0:HD
                        ]
                    )
                    if cc == 1:
                        nc.vector.tensor_copy(vdst, ps[:])
                    else:
                        nc.scalar.activation(vdst, ps[:], AF.Copy)

            # ---- banded attention, 16 heads x 4 query blocks ---------------
            # Scores are computed transposed (s_T[tk, tq]) by swapping the
            # matmul operands, so the probs feed the AV matmul directly as
            # the moving operand -- no transposes.  Chunk c of keys (128 halo
            # rows) serves query blocks c-1 and c; AV for query block qb runs
            # as soon as chunks qb and qb+1 are masked.
            for c in range(NCH):
                jlo, jhi = (128, KW) if c == 0 else ((0, 128) if c == NCH - 1 else (0, KW))
                w = jhi - jlo
                for pr in range(8):
                    for g in range(2):
                        po = 64 * g
                        # one psum tile (one bank) per head: column-split
                        # start=True matmuls into a shared bank are fatal on
                        # HW (overlapping drains + bank-wide has_written clear)
                        ps_sg = psum.tile([128, KW], FP32, name="sc", tag="mm", bufs=4)
                        nc.tensor.matmul(
                            ps_sg[:, 0:w],
                            kT[pr][po : po + 64, 128 * c : 128 * (c + 1)],
                            qT[pr][po : po + 64, 128 * (c - 1) + jlo : 128 * (c - 1) + jhi],
                            start=True,
                            stop=True,
                        )
                        # e = exp(scale * s)
                        e2 = em[c][pr][:, KW * g + jlo : KW * g + jhi]
                        nc.scalar.activation(e2, ps_sg[:, 0:w], AF.Exp, scale=SCALE)
                    # multiply by the binary mask (masked lanes -> exact 0),
                    # both heads of the pair in one op, mask broadcast; a
                    # couple of pairs go to the otherwise-idle gpsimd
                    e3 = em[c][pr][:].rearrange("p (g n) -> p g n", n=KW)[:, :, jlo:jhi]
                    meng = nc.gpsimd if pr >= 5 else nc.vector
                    meng.tensor_tensor(
                        out=e3,
                        in0=e3,
                        in1=mkT[c][:, None, jlo:jhi].broadcast_to([128, 2, w]),
                        op=MUL,
                    )
                if c == 0:
                    continue
                qb = c - 1
                for pr in range(8):
                    for g in range(2):
                        h = 2 * pr + g
                        # token-major out_h[tq, d] + z in col 64 (ones column
                        # of the augmented v): stationary = probs, moving = v
                        ps_av = psum.tile([128, 128], FP32, name="av", tag="ao", bufs=2)
                        nc.tensor.matmul(
                            ps_av[:, 0:VW],
                            em[qb][pr][:, KW * g + 128 : KW * g + KW],
                            vtk[qb][:, VW * h : VW * (h + 1)],
                            start=True,
                            stop=False,
                        )
                        nc.tensor.matmul(
                            ps_av[:, 0:VW],
                            em[qb + 1][pr][:, KW * g : KW * g + 128],
                            vtk[qb + 1][:, VW * h : VW * (h + 1)],
                            start=False,
                            stop=True,
                        )
                        zq = zr[h % 4]
                        nc.vector.reciprocal(zq[:], ps_av[:, HD : HD + 1])
                        if h % 4 == 3:
                            nc.scalar.activation(
                                aoQ[qb][:, HD * h : HD * (h + 1)],
                                ps_av[:, 0:HD],
                                AF.Copy,
                                scale=zq[:],
                            )
                        else:
                            nc.vector.tensor_scalar_mul(
                                aoQ[qb][:, HD * h : HD * (h + 1)],
                                ps_av[:, 0:HD],
                                zq[:],
                            )
                # transpose this query block's normalized attention output to
                # feature-major for the output projection
                for kb in range(8):
                    pt = psum.tile([128, 128], BF, name="tr", tag="tr", bufs=2)
                    nc.tensor.transpose(
                        pt[:], aoQ[qb][:, 128 * kb : 128 * (kb + 1)], ident[:]
                    )
                    if kb >= 6:
                        nc.scalar.activation(
                            aoT[kb][:, 128 * qb : 128 * (qb + 1)], pt[:], AF.Copy
                        )
                    else:
                        nc.vector.tensor_copy(
                            aoT[kb][:, 128 * qb : 128 * (qb + 1)], pt[:]
                        )

            # ---- output projection, feature-major --------------------------
            for cb in range(8):
                ps = psum.tile([128, SPC], FP32, name="mmo", tag="mm", bufs=4)
                for kb in range(8):
                    nc.tensor.matmul(
                        ps[:],
                        wo[kb][:, 128 * cb : 128 * (cb + 1)],
                        aoT[kb][:],
                        start=(kb == 0),
                        stop=(kb == 7),
                    )
                with nc.allow_low_precision(reason="bf16 output store"):
                    nc.scalar.activation(
                        yT[cb][:], ps[:], AF.Identity, bias=bo[:, cb : cb + 1]
                    )
                nc.sync.dma_start(out_d[128 * cb : 128 * (cb + 1), :], yT[cb][:])

    nc.finalize()  # runs bacc passes (wait splitting, reg alloc) + freeze
    return nc


def kernel(x, cantor_coords, W_qkv, b_qkv, W_out, b_out):
    global LAST_RESULTS
    from concourse.bass_utils import run_bass_kernel_spmd

    in_maps, p = _host_prep(x, cantor_coords, W_qkv, b_qkv, W_out, b_out)
    nc = build_nc()
    res = run_bass_kernel_spmd(nc, in_maps, core_ids=list(range(NCORES)))
    LAST_RESULTS = res

    yT = np.concatenate(
        [np.asarray(res.results[t]["out"]).astype(np.float32) for t in range(NCORES)],
        axis=1,
    )  # [DIM, N] in sorted token order
    y = np.empty((N, DIM), dtype=np.float32)
    y[p] = yT.T
    return y.reshape(1, N, DIM)
